# revision 65
# baseline (speedup 1.0000x reference)
"""Trainium2 Bass kernel for causal self-attention with QK RMS-norm + rotary.

Full (unsharded) inputs in, full output out.  Internally sharded over 8
NeuronCores: data parallel on batch (2) x tensor parallel on head groups
(16 heads -> 4 groups of 4).  Each core computes q/k/v for its 4 heads on
its batch, causal flash-style attention, and a partial output projection
(its 512-column slice of Wp's input dim); the host sums the 4 partials per
batch ("all-reduce after proj" done host-side) and adds the output bias.

All matmul operands are bf16 (fp32 PSUM accumulation): the PE streams bf16
and fp32r at the same rate, but LDWEIGHTS of a bf16 stationary tile costs
half of fp32, and DMA traffic halves.  Phase-1 input DMA is spread over all
three queues (weights on sync, x tiles on the ACT queue, cos/sin/ones on
the GpSimd SWDGE) so the x tiles are not starved behind the weight bulk.
Both phases' SBUF pools coexist -- only the PSUM pool swaps at the phase
boundary, so phase 2 waits only for phase 1's last PSUM readers, not an
all-engine drain.  Persistent k^T/v/rk stores are per t-tile and q^T per
query chunk: dependency tracking is per TILE, so early-phase-2 matmuls wait
only on the specific tiles they read.

Per-core pipeline (single Bass program, SPMD over 8 cores):
  Phase 1, per 128-row t-tile: QKV projections with x^T tiles as the
    stationary matmul operand (q/k/v share each weight load); rotary applied
    to raw q/k straight out of PSUM (rotation commutes with RMS-norm's
    per-row scale), rotated tiles written bf16; per-head squared-norm via
    multiply+reduce; q normalized in place; k's norm and the 0.12 score
    scale folded into one Sqrt+reciprocal that writes the per-(t,head) exp
    scale directly; q^T/k^T via bf16 PE transposes (1 cycle/row) in the
    baseline's K/V-then-trailing-Q software pipeline; v drained by the
    (phase-1-idle) ACT engine.
  Phase 2, per 512-column query chunk, per head pair: scores^T =
    k_tile^T-block @ q^T computed [j,i]-transposed so the softmax
    denominator comes from a ones-stationary matmul (every PSUM partition =
    sum_j p) and attn@v needs no transpose of p; exp on ScalarE (bf16 out)
    prefetched one j-step ahead; causal mask on diagonal blocks via GpSimd
    affine_select (upper-triangle blocks never computed); y^T accumulated in
    PSUM over j-tiles, then drained fast (y on DVE, l on ACT) to release
    the accumulators before the slow reciprocal+normalize run SBUF-side at
    bf16 into four per-128-column y tiles emitted in the projection's
    consumption order (deps are per tile, so proj's LDWEIGHTS waits cover
    one 128-col reciprocal, not the whole chain); the previous chunk's
    projection is emitted after each pair's sixth j-step so the PE chews on
    it mid-pair; the final flush normalizes straight from PSUM and drains
    on the by-then-idle ACT engine.
"""

import os
import sys

import numpy as np

try:
    import concourse.bass as bass
except ImportError:  # fall back to the repo checkout baked into the image
    for _p in ("/opt/trn_rl_repo", "/root/.axon_site/_ro/trn_rl_repo"):
        if os.path.isdir(_p) and _p not in sys.path:
            sys.path.append(_p)
    import concourse.bass as bass

import concourse.mybir as mybir
import concourse.tile as tile
from concourse.bass_utils import run_bass_kernel_spmd
from concourse.masks import make_identity
from concourse.vector_clock import ScopedClock

F32 = mybir.dt.float32
BF16 = mybir.dt.bfloat16
AF = mybir.ActivationFunctionType

DIM = 2048
HEAD_DIM = 128
NUM_HEADS = 16
B, T = 2, 2048
EPS = 1.1920929e-07
SCALE = 0.12

NCORES = 8
HG = 4                    # heads per core
GD = HG * HEAD_DIM        # 512: per-core q/k/v width and Wp input slice
NT = T // 128             # 16 t-tiles
ND = DIM // 128           # 16 contraction tiles
NI = T // 512             # 4 query chunks
P = 128


class _TC(tile.TileContext):
    """TileContext whose final drain splits its semaphore waits across
    single-wait NOPs -- the walrus build in this image rejects CTRL
    instructions carrying 3+ sync waits ("Too many sync wait commands")."""

    def _drain_and_barrier(self, tick_clock, wait_clock):
        probe = self.nc.sync.nop(nofuse=True)
        wait_clock.add_sem_waits(probe.ins, ScopedClock({None: tick_clock.global_clock}))
        si = probe.ins.sync_info
        waits = list(si.on_wait) if si and si.on_wait else []
        if si is not None and si.on_wait:
            del si.on_wait[1:]
        for w in waits[1:]:
            nop = self.nc.sync.nop(nofuse=True)
            nsi = nop.ins.sync_info
            if nsi is None:
                nop.ins.sync_info = mybir.SyncInfo(on_wait=[w], on_update=[])
            else:
                nsi.on_wait.append(w)
        self.nc.sync.drain()
        self.nc.all_engine_barrier()
        assert self.sems is not None
        popped = self.nc._tile_sem_poison_stack.pop()
        assert popped is self._sem_poison
        self.nc.clear_and_free_semaphores(list(self.sems.allocated().values()))
        self.nc.all_engine_barrier()


_MAX_WAITS = 1


def _split_excess_waits(nc, maxw=_MAX_WAITS):
    """The walrus build in this image rejects instructions with >1 sync
    waits; spill extra waits onto NoOps inserted just before the offender
    on the same engine (all waits are preconditions, so order is free)."""
    n = 0
    for f in nc.m.functions:
        for bb in f.blocks:
            out = []
            for inst in bb.instructions:
                si = inst.sync_info
                waits = list(si.on_wait) if si and si.on_wait else []
                if len(waits) > maxw:
                    extra = waits[:-maxw]
                    del si.on_wait[: len(extra)]
                    for i in range(0, len(extra), maxw):
                        n += 1
                        nop = mybir.InstNoOp(name=f"I-wsplit-{n}-{inst.name}",
                                             ins=[], outs=[])
                        nop.engine = inst.engine
                        nop.sync_info = mybir.SyncInfo(
                            on_wait=extra[i:i + maxw], on_update=[])
                        out.append(nop)
                out.append(inst)
            bb.instructions[:] = out


def _build_nc(has_qkv_bias: bool):
    nc = bass.Bass("TRN2", target_bir_lowering=False, debug=False, num_devices=NCORES)

    xt = nc.dram_tensor("xt", [DIM, T], BF16, kind="ExternalInput")
    wqt = nc.dram_tensor("wqt", [DIM, GD], BF16, kind="ExternalInput")
    wkt = nc.dram_tensor("wkt", [DIM, GD], BF16, kind="ExternalInput")
    wvt = nc.dram_tensor("wvt", [DIM, GD], BF16, kind="ExternalInput")
    wpt = nc.dram_tensor("wpt", [GD, DIM], BF16, kind="ExternalInput")
    cosb = nc.dram_tensor("cosb", [T, GD], F32, kind="ExternalInput")
    onesd = nc.dram_tensor("onesd", [P, P], BF16, kind="ExternalInput")
    sinb = nc.dram_tensor("sinb", [T, GD], F32, kind="ExternalInput")
    if has_qkv_bias:
        bq = nc.dram_tensor("bq", [GD], F32, kind="ExternalInput")
        bk = nc.dram_tensor("bk", [GD], F32, kind="ExternalInput")
        bv = nc.dram_tensor("bv", [GD], F32, kind="ExternalInput")
    out = nc.dram_tensor("out", [T, DIM], F32, kind="ExternalOutput")

    xt_v = xt.rearrange("(do p) t -> p do t", p=P)      # [128, 16, 2048]
    wqt_v = wqt.rearrange("(do p) o -> p do o", p=P)    # [128, 16, 512]
    wkt_v = wkt.rearrange("(do p) o -> p do o", p=P)
    wvt_v = wvt.rearrange("(do p) o -> p do o", p=P)
    wpt_v = wpt.rearrange("(co p) o -> p co o", p=P)    # [128, 4, 2048]

    with _TC(nc) as tc:
        with (
            tc.tile_pool(name="const", bufs=1) as constp,
            tc.tile_pool(name="persist", bufs=1) as persist,
        ):
            ones_t = constp.tile([P, P], BF16)
            identity = constp.tile([P, P], BF16)
            make_identity(nc, identity)
            eps_q = constp.tile([P, 1], F32)
            nc.vector.memset(eps_q, EPS)
            eps_k = constp.tile([P, 1], F32)
            nc.vector.memset(eps_k, EPS / (SCALE * SCALE))
            if has_qkv_bias:
                bias_b = constp.tile([P, 3, GD], F32)
                for bi, bten in enumerate((bq, bk, bv)):
                    bcast = bass.AP(tensor=bten.tensor, offset=bten.offset,
                                    ap=[[0, P]] + list(bten.ap))
                    nc.sync.dma_start(out=bias_b[:, bi, :], in_=bcast)

            # Per-tile / per-chunk persistent stores: deps are tracked per
            # TILE, so chunk-0 attention (which needs only t-tiles 0-3) can
            # issue as soon as those land instead of waiting for all of
            # phase 1 to finish writing one big resident tensor.
            v_t = {tt: persist.tile([P, GD], BF16, name=f"v{tt}")
                   for tt in range(NT)}                  # v, natural [t, h*128]
            rk_t = {tt: persist.tile([P, HG], F32, name=f"rk{tt}")
                    for tt in range(NT)}                 # 0.12/rms(k)
            kt_t = {tt: persist.tile([P, HG, P], BF16, name=f"kt{tt}")
                    for tt in range(NT)}                 # k^T [c, h, t]
            qt_c = {c: persist.tile([P, 4, HG, P], BF16, name=f"qtc{c}")
                    for c in range(NI)}                  # q^T per query chunk

            # ---------------- Phase 1: QKV + rotary + norms + transposes ------
            # K/V for t-tile i and Q for t-tile i-1 per iteration: Q trails
            # one tile so the PE starts on K/V as soon as the first weight
            # chunks land instead of waiting for all three weight matrices.
            # Both phases' SBUF pools coexist (no SBUF-reuse barrier at the
            # phase boundary); only the PSUM pool is swapped, so phase-2's
            # first matmuls wait just for phase-1's last PSUM readers, not
            # for every engine to drain.
            with (
                tc.tile_pool(name="wqkv", bufs=1) as wpool,
                tc.tile_pool(name="ph1", bufs=3) as ph1,
                tc.tile_pool(name="ph1t", bufs=2) as ph1t,
                tc.tile_pool(name="ph2w", bufs=1) as ph2w,
                tc.tile_pool(name="ph2", bufs=2) as ph2,
            ):
              with tc.tile_pool(name="pp1", bufs=1, space="PSUM") as pp1:
                wq_sb = wpool.tile([P, ND, GD], BF16)
                wk_sb = wpool.tile([P, ND, GD], BF16)
                wv_sb = wpool.tile([P, ND, GD], BF16)

                acts = {}

                # DMA queue split: weights on the sync queue, x tiles on the
                # ACT queue, cos/sin (+ones) on the GpSimd SWDGE -- one queue
                # can't sustain all 23MB of phase-1 input without starving
                # the x tiles behind the weight bulk.
                def load_acts(tt):
                    tsl = slice(tt * P, (tt + 1) * P)
                    if tt == 0:
                        # Separate TILES (deps are tracked per tile, not per
                        # slice): the first K matmul's LDWEIGHTS only waits
                        # for the small head tile, not the full 512KB load.
                        xh = ph1.tile([P, 2, P], BF16, tag="x0h", bufs=1,
                                      name="xtile0h")
                        # first thing on the sync queue: the first matmul's
                        # stationary data
                        nc.sync.dma_start(out=xh, in_=xt_v[:, :2, tsl])
                        xt_t = ph1.tile([P, ND - 2, P], BF16, tag="x0t",
                                        bufs=1, name="xtile0t")
                        nc.scalar.dma_start(out=xt_t, in_=xt_v[:, 2:, tsl])

                        def xf(d, xh=xh, xt_t=xt_t):
                            return xh[:, d, :] if d < 2 else xt_t[:, d - 2, :]
                    else:
                        xtile = ph1.tile([P, ND, P], BF16, tag="xtile",
                                         name=f"xtile{tt}")
                        nc.scalar.dma_start(out=xtile, in_=xt_v[:, :, tsl])

                        def xf(d, xtile=xtile):
                            return xtile[:, d, :]
                    ctile = ph1.tile([P, GD], F32, tag="ctile", name=f"ctile{tt}")
                    stile = ph1.tile([P, GD], F32, tag="stile", name=f"stile{tt}")
                    nc.gpsimd.dma_start(out=ctile,
                                        in_=cosb[tt * P:(tt + 1) * P, :])
                    nc.gpsimd.dma_start(out=stile,
                                        in_=sinb[tt * P:(tt + 1) * P, :])
                    acts[tt] = (xf, ctile, stile)

                # First K/V weight chunks land before the bulk x tile so the
                # PE's first matmul isn't gated on the largest DMA; the rest
                # is d-interleaved so each K/V contraction step can begin as
                # its chunks arrive.  Q chunks trail by design.  The ones
                # tile (first needed in phase 2) loads after the hot path.
                load_acts(0)
                nc.sync.dma_start(out=wk_sb[:, 0, :], in_=wkt_v[:, 0, :])
                nc.sync.dma_start(out=wv_sb[:, 0, :], in_=wvt_v[:, 0, :])
                for d in range(ND):
                    if d > 0:
                        # first few K/V chunks ride the near-idle gpsimd
                        # queue so the startup matmuls aren't gated on one
                        # queue draining the whole weight bulk
                        wq_ = nc.gpsimd if d < 4 else nc.sync
                        wq_.dma_start(out=wk_sb[:, d, :], in_=wkt_v[:, d, :])
                        wq_.dma_start(out=wv_sb[:, d, :], in_=wvt_v[:, d, :])
                    nc.sync.dma_start(out=wq_sb[:, d, :], in_=wqt_v[:, d, :])
                nc.gpsimd.dma_start(out=ones_t, in_=onesd[:, :])

                def qk_dve(which, src, tt):
                    """Rotary + rms stats on DVE/ACT, then DMA-transpose the
                    rotated bf16 tile into the resident q^T/k^T store."""
                    _, ctile, stile = acts[tt]
                    u = ph1t.tile([P, HG, 2, 64], F32, tag="u")
                    w = ph1t.tile([P, HG, 2, 64], F32, tag="w")
                    nc.vector.tensor_mul(u.rearrange("p h x y -> p (h x y)"), src, ctile)
                    nc.vector.tensor_mul(w.rearrange("p h x y -> p (h x y)"), src, stile)
                    rot = ph1t.tile([P, HG, P], BF16, tag=which + "rot")
                    r3 = rot.rearrange("p h (x y) -> p h x y", x=2)
                    nc.vector.tensor_add(r3[:, :, 0, :], u[:, :, 0, :], w[:, :, 1, :])
                    nc.vector.tensor_sub(r3[:, :, 1, :], u[:, :, 1, :], w[:, :, 0, :])

                    ms = ph1t.tile([P, HG], F32, tag=which + "ms")
                    sq = ph1t.tile([P, HG, P], BF16, tag="sq")
                    rflat = rot.rearrange("p h c -> p (h c)")
                    nc.vector.tensor_mul(
                        sq.rearrange("p h c -> p (h c)"), rflat, rflat)
                    nc.vector.reduce_sum(out=ms, in_=sq,
                                         axis=mybir.AxisListType.X)
                    srt = ph1t.tile([P, HG], F32, tag=which + "srt")
                    if which == "q":
                        nc.scalar.activation(out=srt, in_=ms, func=AF.Sqrt,
                                             scale=1.0 / HEAD_DIM,
                                             bias=eps_q[:, 0:1])
                        rstd = ph1t.tile([P, HG], F32, tag="qrstd")
                        nc.vector.reciprocal(out=rstd, in_=srt)
                        for h in range(HG):
                            # all-SBUF op on the q-transpose critical chain:
                            # GpSimd (idle in phase 1) instead of the busy DVE
                            nc.gpsimd.tensor_scalar_mul(
                                rot[:, h, :], rot[:, h, :], rstd[:, h:h + 1])
                    else:
                        # 0.12/rms(k) folded in: 0.12/sqrt(z) = 1/sqrt(z/0.0144)
                        nc.scalar.activation(
                            out=srt, in_=ms, func=AF.Sqrt,
                            scale=1.0 / (HEAD_DIM * SCALE * SCALE),
                            bias=eps_k[:, 0:1])
                        nc.vector.reciprocal(out=rk_t[tt], in_=srt)
                    return rot

                def qk_transpose(which, rot, tt):
                    def dst(h):
                        if which == "k":
                            return kt_t[tt][:, h, :]
                        return qt_c[tt // 4][:, tt % 4, h, :]
                    for h in range(HG):
                        ptr = pp1.tile([P, P], BF16, tag="ptr", bufs=2)
                        nc.tensor.transpose(ptr, rot[:, h, :], identity)
                        nc.vector.tensor_copy(out=dst(h), in_=ptr)

                # Software pipeline: per iteration i the PE runs K/V(i) then
                # Q(i-1), then transposes k(i) (whose DVE chain overlapped
                # the Q matmuls) and q(i-2) (chain finished an iteration ago).
                qrots = {}
                for i in range(NT + 2):
                    if 2 <= i:
                        qk_transpose("q", qrots.pop(i - 2), i - 2)
                    if i < NT:
                        if i + 1 < NT:
                            load_acts(i + 1)
                        xf = acts[i][0]
                        ps_k = pp1.tile([P, GD], F32, tag="psk", bufs=2)
                        ps_v = pp1.tile([P, GD], F32, tag="psv", bufs=2)
                        for d in range(ND):
                            st, sp = d == 0, d == ND - 1
                            nc.tensor.matmul(ps_k, xf(d), wk_sb[:, d, :],
                                             start=st, stop=sp)
                            nc.tensor.matmul(ps_v, xf(d), wv_sb[:, d, :],
                                             start=st, stop=sp)
                        if has_qkv_bias:
                            kb = ph1t.tile([P, GD], F32, tag="kb")
                            nc.vector.tensor_add(kb, ps_k, bias_b[:, 1, :])
                            nc.vector.tensor_add(v_sb[:, i, :], ps_v,
                                                 bias_b[:, 2, :])
                            src_k = kb
                        else:
                            # GpSimd can't read PSUM; ACT is idle in phase 1
                            nc.scalar.activation(out=v_sb[:, i, :], in_=ps_v,
                                                 func=AF.Copy)
                            src_k = ps_k
                        krot = qk_dve("k", src_k, i)
                    if 1 <= i <= NT:
                        tt = i - 1
                        xf = acts[tt][0]
                        ps_q = pp1.tile([P, GD], F32, tag="psq", bufs=2)
                        for d in range(ND):
                            nc.tensor.matmul(ps_q, xf(d), wq_sb[:, d, :],
                                             start=d == 0, stop=d == ND - 1)
                        if has_qkv_bias:
                            qb = ph1t.tile([P, GD], F32, tag="qb")
                            nc.vector.tensor_add(qb, ps_q, bias_b[:, 0, :])
                            src_q = qb
                        else:
                            src_q = ps_q
                        qrots[tt] = qk_dve("q", src_q, tt)
                    if i < NT:
                        qk_transpose("k", krot, i)

              # -------------- Phase 2: attention + projection ---------------
              with tc.tile_pool(name="pp2", bufs=1, space="PSUM") as pp2:
                wp_sb = ph2w.tile([P, HG, DIM], BF16)

                def ysl(y_tiles, co, it):
                    t = y_tiles[co // 2]
                    if isinstance(t, dict):   # final pair: per-it-chunk tiles
                        return t[it][:, co % 2, :]
                    return t[:, co % 2, it * P:(it + 1) * P]

                def proj_half(ic, y_tiles, half, drain_act=False):
                    for it in (half, 2 + half):
                        for dc in range(4):
                            ps_o = pp2.tile([P, 512], F32, tag="po", bufs=2)
                            for co in range(HG):
                                nc.tensor.matmul(
                                    ps_o,
                                    ysl(y_tiles, co, it),
                                    wp_sb[:, co, dc * 512:(dc + 1) * 512],
                                    start=co == 0, stop=co == HG - 1)
                            o_sb = ph2.tile([P, 512], F32, tag="o", bufs=3)
                            if drain_act:
                                # final flush: DVE is busy with the last
                                # normalize; ACT is idle by now
                                nc.scalar.activation(out=o_sb, in_=ps_o,
                                                     func=AF.Copy)
                            else:
                                nc.vector.tensor_copy(out=o_sb, in_=ps_o)
                            nc.sync.dma_start(
                                out=out[(4 * ic + it) * P:(4 * ic + it + 1) * P,
                                        dc * 512:(dc + 1) * 512],
                                in_=o_sb)

                prev = None
                for ic in range(NI):
                    if ic == 0:
                        nc.sync.dma_start(out=wp_sb, in_=wpt_v)
                    qT = {h: qt_sb[:, 4 * ic:4 * ic + 4, h, :] for h in range(HG)}
                    # one y tile per head PAIR: deps are tracked per tile, so
                    # proj's LDWEIGHTS on heads 0-1 isn't gated on the last
                    # pair's normalize chain
                    y_tiles = {}
                    nj = 4 * (ic + 1)
                    # Heads in interleaved pairs with scores+exp prefetched one
                    # j-step ahead: by the time the PE reaches l/pV of step j,
                    # the exp (and diagonal-block mask) of step j has had four
                    # matmuls' worth of time to finish.  Half of the previous
                    # chunk's projection is emitted at each pair boundary so
                    # the PE chews on it exactly where it would otherwise wait
                    # for the pair's PSUM accumulators to drain.
                    for hp in range(HG // 2):
                        hs = (2 * hp, 2 * hp + 1)
                        ps_ys = {h: pp2.tile([P, 512], F32, tag="py", bufs=2,
                                             name=f"ps_y{h}") for h in hs}
                        ps_ls = {h: pp2.tile([P, 512], F32, tag="pl", bufs=2,
                                             name=f"ps_l{h}") for h in hs}

                        def sc_exp(jt):
                            pt = ph2.tile([P, 2, 512], BF16, tag="p", bufs=3)
                            for hi, h in enumerate(hs):
                                ps_s = pp2.tile([P, 512], F32, tag="ps", bufs=2)
                                nc.tensor.matmul(
                                    ps_s, kt_sb[:, jt, h, :],
                                    qT[h], start=True, stop=True)
                                nc.scalar.activation(
                                    out=pt[:, hi, :], in_=ps_s, func=AF.Exp,
                                    scale=rk_sb[:, jt, h:h + 1])
                                if jt >= 4 * ic:
                                    nc.gpsimd.affine_select(
                                        out=pt[:, hi, :], in_=pt[:, hi, :],
                                        pattern=[[1, 512]], channel_multiplier=-1,
                                        base=-P * (jt - 4 * ic),
                                        compare_op=mybir.AluOpType.is_ge, fill=0.0)
                            return pt

                        pts = sc_exp(0)
                        for jt in range(nj):
                            nxt = None
                            if jt + 1 < nj:
                                nxt = sc_exp(jt + 1)
                            st, sp = jt == 0, jt == nj - 1
                            for hi, h in enumerate(hs):
                                nc.tensor.matmul(ps_ls[h], ones_t,
                                                 pts[:, hi, :], start=st, stop=sp)
                                nc.tensor.matmul(
                                    ps_ys[h], v_sb[:, jt, h * P:(h + 1) * P],
                                    pts[:, hi, :], start=st, stop=sp)
                            pts = nxt
                            # Previous chunk's projection lands after this
                            # pair's fourth j-step: the PE runs it while this
                            # pair streams, and the previous pair's
                            # recip+normalize chain (DVE, ~4us) has three full
                            # j-steps of slack before proj's LDWEIGHTS needs
                            # the chunk's last y_sb slice.
                            if jt == 3 and prev is not None:
                                proj_half(prev[0], prev[1], hp)
                        # Fast ACT copies release all four PSUM banks within
                        # ~1us of the pair's last matmul; the slow reciprocal
                        # and the normalize run SBUF-side on DVE with a full
                        # pair of slack before y_sb is consumed by proj.
                        # Quick bf16 drains free all four PSUM banks fast:
                        # y on DVE (idle at pair end), l on ACT; the slow
                        # reciprocal + normalize run SBUF-side afterwards,
                        # off every matmul's critical path (per-pair y tiles
                        # + proj placement give them ~2 j-steps of slack).
                        linv = ph2.tile([P, 2, 512], BF16, tag="linv")
                        last = ic == NI - 1 and hp == HG // 2 - 1
                        if last:
                            # Nothing follows: normalize straight from PSUM,
                            # in four per-it-chunk tiles emitted in the proj
                            # flush's consumption order (it = 0,2,1,3), so
                            # the flush starts after the first 128-col chunk
                            # instead of after the whole recip chain.
                            yd = {}
                            with nc.allow_low_precision(reason="bf16 y"):
                                for c in (0, 2, 1, 3):
                                    yt = ph2.tile([P, 2, P], BF16, tag="yf",
                                                  bufs=4, name=f"yf{c}")
                                    yd[c] = yt
                                    sl = slice(c * P, (c + 1) * P)
                                    for hi, h in enumerate(hs):
                                        nc.vector.reciprocal(
                                            out=linv[:, hi, sl],
                                            in_=ps_ls[h][:, sl])
                                        nc.vector.tensor_mul(
                                            yt[:, hi, :], ps_ys[h][:, sl],
                                            linv[:, hi, sl])
                            y_tiles[hp] = yd
                        else:
                            y_raw = ph2.tile([P, 2, 512], BF16, tag="y_raw")
                            l_sb = ph2.tile([P, 2, 512], BF16, tag="l_sb")
                            for hi, h in enumerate(hs):
                                nc.vector.tensor_copy(out=y_raw[:, hi, :],
                                                      in_=ps_ys[h])
                                nc.scalar.activation(out=l_sb[:, hi, :],
                                                     in_=ps_ls[h],
                                                     func=AF.Copy)
                            # fine-grained: four per-it-chunk y tiles in the
                            # proj flush's consumption order, so proj's
                            # per-tile waits cover one 128-col recip, not
                            # the whole chain
                            yd = {}
                            with nc.allow_low_precision(
                                    reason="1/l and y at bf16; y is consumed "
                                           "as bf16 anyway"):
                                for c in (0, 2, 1, 3):
                                    yt = ph2.tile([P, 2, P], BF16,
                                                  tag=f"y{hp}c", bufs=8,
                                                  name=f"y{ic}_{hp}_{c}")
                                    yd[c] = yt
                                    sl = slice(c * P, (c + 1) * P)
                                    for hi, h in enumerate(hs):
                                        nc.vector.reciprocal(
                                            out=linv[:, hi, sl],
                                            in_=l_sb[:, hi, sl])
                                        nc.vector.tensor_mul(
                                            yt[:, hi, :],
                                            y_raw[:, hi, sl],
                                            linv[:, hi, sl])
                            y_tiles[hp] = yd
                    prev = (ic, y_tiles)
                proj_half(prev[0], prev[1], 0, drain_act=True)
                proj_half(prev[0], prev[1], 1, drain_act=True)
    _split_excess_waits(nc)
    return nc


_NC_CACHE = {}
_RUN_KWARGS = {}      # test harness hook: e.g. {"trace": True}
_LAST_RESULT = None   # BassKernelResults of the most recent run


def _rotary_tables():
    freq = (1.0 / 1024.0) ** np.linspace(0.0, 1.0, HEAD_DIM // 4, dtype=np.float32)
    freq = np.concatenate([freq, np.zeros(HEAD_DIM // 4, np.float32)])
    theta = np.arange(T, dtype=np.float32)[:, None] * freq[None, :]     # [T, 64]
    cos = np.cos(theta).astype(np.float32)
    sin = np.sin(theta).astype(np.float32)
    cosb = np.tile(np.concatenate([cos, cos], axis=1), (1, HG))          # [T, 512]
    sinb = np.tile(np.concatenate([sin, sin], axis=1), (1, HG))
    return np.ascontiguousarray(cosb), np.ascontiguousarray(sinb)


def kernel(x, Wq, bq, Wk, bk, Wv, bv, Wp, bp):
    import ml_dtypes
    BF = ml_dtypes.bfloat16

    x = np.asarray(x, np.float32)
    Wq, Wk, Wv, Wp = (np.asarray(a, np.float32) for a in (Wq, Wk, Wv, Wp))
    bq, bk, bv, bp = (np.asarray(a, np.float32) for a in (bq, bk, bv, bp))

    has_bias = bool(np.any(bq) or np.any(bk) or np.any(bv))
    if has_bias not in _NC_CACHE:
        _NC_CACHE[has_bias] = _build_nc(has_bias)
    nc = _NC_CACHE[has_bias]

    cosb, sinb = _rotary_tables()
    in_maps = []
    for c in range(NCORES):
        b, g = divmod(c, NCORES // B)
        sl = slice(g * GD, (g + 1) * GD)
        m = {
            "xt": np.ascontiguousarray(x[b].T.astype(BF)),
            "wqt": np.ascontiguousarray(Wq[sl, :].T.astype(BF)),
            "wkt": np.ascontiguousarray(Wk[sl, :].T.astype(BF)),
            "wvt": np.ascontiguousarray(Wv[sl, :].T.astype(BF)),
            "wpt": np.ascontiguousarray(Wp[:, sl].T.astype(BF)),
            "cosb": cosb,
            "sinb": sinb,
            "onesd": np.ones((P, P), BF),
        }
        if has_bias:
            m["bq"] = np.ascontiguousarray(bq[sl])
            m["bk"] = np.ascontiguousarray(bk[sl])
            m["bv"] = np.ascontiguousarray(bv[sl])
        in_maps.append(m)

    res = run_bass_kernel_spmd(nc, in_maps, list(range(NCORES)), **_RUN_KWARGS)
    global _LAST_RESULT
    _LAST_RESULT = res
    out = np.zeros((B, T, DIM), np.float32)
    for c in range(NCORES):
        out[c // (NCORES // B)] += res.results[c]["out"]
    out += bp[None, None, :]
    return out


# revision 66
# speedup vs baseline: 1.3410x; 1.3410x over previous
"""Trainium2 Bass kernel for causal self-attention with QK RMS-norm + rotary.

Full (unsharded) inputs in, full output out.  Internally sharded over 8
NeuronCores: data parallel on batch (2) x tensor parallel on head groups
(16 heads -> 4 groups of 4).  Each core computes q/k/v for its 4 heads on
its batch, causal flash-style attention, and a partial output projection
(its 512-column slice of Wp's input dim); the host sums the 4 partials per
batch ("all-reduce after proj" done host-side) and adds the output bias.

All matmul operands are bf16 (fp32 PSUM accumulation): the PE streams bf16
and fp32r at the same rate, but LDWEIGHTS of a bf16 stationary tile costs
half of fp32, and DMA traffic halves.  Phase-1 input DMA is spread over all
three queues (weights on sync, x tiles on the ACT queue, cos/sin/ones on
the GpSimd SWDGE) so the x tiles are not starved behind the weight bulk.
Both phases' SBUF pools coexist -- only the PSUM pool swaps at the phase
boundary, so phase 2 waits only for phase 1's last PSUM readers, not an
all-engine drain.  Persistent k^T/v/rk stores are per t-tile and q^T per
query chunk: dependency tracking is per TILE, so early-phase-2 matmuls wait
only on the specific tiles they read.

Per-core pipeline (single Bass program, SPMD over 8 cores):
  Phase 1, per 128-row t-tile: QKV projections with x^T tiles as the
    stationary matmul operand (q/k/v share each weight load); rotary applied
    to raw q/k straight out of PSUM (rotation commutes with RMS-norm's
    per-row scale), rotated tiles written bf16; per-head squared-norm via
    multiply+reduce; q normalized in place; k's norm and the 0.12 score
    scale folded into one Sqrt+reciprocal that writes the per-(t,head) exp
    scale directly; q^T/k^T via bf16 PE transposes (1 cycle/row) in the
    baseline's K/V-then-trailing-Q software pipeline; v drained by the
    (phase-1-idle) ACT engine.
  Phase 2, per 512-column query chunk, per head pair: scores^T =
    k_tile^T-block @ q^T computed [j,i]-transposed so the softmax
    denominator comes from a ones-stationary matmul (every PSUM partition =
    sum_j p) and attn@v needs no transpose of p; exp on ScalarE (bf16 out)
    prefetched one j-step ahead; causal mask on diagonal blocks via GpSimd
    affine_select (upper-triangle blocks never computed); y^T accumulated in
    PSUM over j-tiles, then drained fast (y on DVE, l on ACT) to release
    the accumulators before the slow reciprocal+normalize run SBUF-side at
    bf16 into four per-128-column y tiles emitted in the projection's
    consumption order (deps are per tile, so proj's LDWEIGHTS waits cover
    one 128-col reciprocal, not the whole chain); the previous chunk's
    projection is emitted after each pair's sixth j-step so the PE chews on
    it mid-pair; the final flush normalizes straight from PSUM and drains
    on the by-then-idle ACT engine.
"""

import os
import sys

import numpy as np

try:
    import concourse.bass as bass
except ImportError:  # fall back to the repo checkout baked into the image
    for _p in ("/opt/trn_rl_repo", "/root/.axon_site/_ro/trn_rl_repo"):
        if os.path.isdir(_p) and _p not in sys.path:
            sys.path.append(_p)
    import concourse.bass as bass

import concourse.mybir as mybir
import concourse.tile as tile
from concourse.bass_utils import run_bass_kernel_spmd
from concourse.masks import make_identity
from concourse.vector_clock import ScopedClock

F32 = mybir.dt.float32
BF16 = mybir.dt.bfloat16
AF = mybir.ActivationFunctionType

DIM = 2048
HEAD_DIM = 128
NUM_HEADS = 16
B, T = 2, 2048
EPS = 1.1920929e-07
SCALE = 0.12

NCORES = 8
HG = 4                    # heads per core
GD = HG * HEAD_DIM        # 512: per-core q/k/v width and Wp input slice
NT = T // 128             # 16 t-tiles
ND = DIM // 128           # 16 contraction tiles
NI = T // 512             # 4 query chunks
P = 128


class _TC(tile.TileContext):
    """TileContext whose final drain splits its semaphore waits across
    single-wait NOPs -- the walrus build in this image rejects CTRL
    instructions carrying 3+ sync waits ("Too many sync wait commands")."""

    def _drain_and_barrier(self, tick_clock, wait_clock):
        probe = self.nc.sync.nop(nofuse=True)
        wait_clock.add_sem_waits(probe.ins, ScopedClock({None: tick_clock.global_clock}))
        si = probe.ins.sync_info
        waits = list(si.on_wait) if si and si.on_wait else []
        if si is not None and si.on_wait:
            del si.on_wait[1:]
        for w in waits[1:]:
            nop = self.nc.sync.nop(nofuse=True)
            nsi = nop.ins.sync_info
            if nsi is None:
                nop.ins.sync_info = mybir.SyncInfo(on_wait=[w], on_update=[])
            else:
                nsi.on_wait.append(w)
        self.nc.sync.drain()
        self.nc.all_engine_barrier()
        assert self.sems is not None
        popped = self.nc._tile_sem_poison_stack.pop()
        assert popped is self._sem_poison
        self.nc.clear_and_free_semaphores(list(self.sems.allocated().values()))
        self.nc.all_engine_barrier()


_MAX_WAITS = 1


def _split_excess_waits(nc, maxw=_MAX_WAITS):
    """The walrus build in this image rejects instructions with >1 sync
    waits; spill extra waits onto NoOps inserted just before the offender
    on the same engine (all waits are preconditions, so order is free)."""
    n = 0
    for f in nc.m.functions:
        for bb in f.blocks:
            out = []
            for inst in bb.instructions:
                si = inst.sync_info
                waits = list(si.on_wait) if si and si.on_wait else []
                if len(waits) > maxw:
                    extra = waits[:-maxw]
                    del si.on_wait[: len(extra)]
                    for i in range(0, len(extra), maxw):
                        n += 1
                        nop = mybir.InstNoOp(name=f"I-wsplit-{n}-{inst.name}",
                                             ins=[], outs=[])
                        nop.engine = inst.engine
                        nop.sync_info = mybir.SyncInfo(
                            on_wait=extra[i:i + maxw], on_update=[])
                        out.append(nop)
                out.append(inst)
            bb.instructions[:] = out


def _build_nc(has_qkv_bias: bool):
    nc = bass.Bass("TRN2", target_bir_lowering=False, debug=False, num_devices=NCORES)

    xt = nc.dram_tensor("xt", [DIM, T], BF16, kind="ExternalInput")
    wqt = nc.dram_tensor("wqt", [DIM, GD], BF16, kind="ExternalInput")
    wkt = nc.dram_tensor("wkt", [DIM, GD], BF16, kind="ExternalInput")
    wvt = nc.dram_tensor("wvt", [DIM, GD], BF16, kind="ExternalInput")
    wpt = nc.dram_tensor("wpt", [GD, DIM], BF16, kind="ExternalInput")
    cosb = nc.dram_tensor("cosb", [T, GD], F32, kind="ExternalInput")
    onesd = nc.dram_tensor("onesd", [P, P], BF16, kind="ExternalInput")
    sinb = nc.dram_tensor("sinb", [T, GD], F32, kind="ExternalInput")
    if has_qkv_bias:
        bq = nc.dram_tensor("bq", [GD], F32, kind="ExternalInput")
        bk = nc.dram_tensor("bk", [GD], F32, kind="ExternalInput")
        bv = nc.dram_tensor("bv", [GD], F32, kind="ExternalInput")
    out = nc.dram_tensor("out", [T, DIM], F32, kind="ExternalOutput")

    xt_v = xt.rearrange("(do p) t -> p do t", p=P)      # [128, 16, 2048]
    wqt_v = wqt.rearrange("(do p) o -> p do o", p=P)    # [128, 16, 512]
    wkt_v = wkt.rearrange("(do p) o -> p do o", p=P)
    wvt_v = wvt.rearrange("(do p) o -> p do o", p=P)
    wpt_v = wpt.rearrange("(co p) o -> p co o", p=P)    # [128, 4, 2048]

    with _TC(nc) as tc:
        with (
            tc.tile_pool(name="const", bufs=1) as constp,
            tc.tile_pool(name="persist", bufs=1) as persist,
        ):
            ones_t = constp.tile([P, P], BF16)
            identity = constp.tile([P, P], BF16)
            make_identity(nc, identity)
            eps_q = constp.tile([P, 1], F32)
            nc.vector.memset(eps_q, EPS)
            eps_k = constp.tile([P, 1], F32)
            nc.vector.memset(eps_k, EPS / (SCALE * SCALE))
            if has_qkv_bias:
                bias_b = constp.tile([P, 3, GD], F32)
                for bi, bten in enumerate((bq, bk, bv)):
                    bcast = bass.AP(tensor=bten.tensor, offset=bten.offset,
                                    ap=[[0, P]] + list(bten.ap))
                    nc.sync.dma_start(out=bias_b[:, bi, :], in_=bcast)

            # Per-tile / per-chunk persistent stores: deps are tracked per
            # TILE, so chunk-0 attention (which needs only t-tiles 0-3) can
            # issue as soon as those land instead of waiting for all of
            # phase 1 to finish writing one big resident tensor.
            v_t = {tt: persist.tile([P, GD], BF16, name=f"v{tt}")
                   for tt in range(NT)}                  # v, natural [t, h*128]
            rk_t = {tt: persist.tile([P, HG], F32, name=f"rk{tt}")
                    for tt in range(NT)}                 # 0.12/rms(k)
            kt_t = {tt: persist.tile([P, HG, P], BF16, name=f"kt{tt}")
                    for tt in range(NT)}                 # k^T [c, h, t]
            qt_c = {c: persist.tile([P, 4, HG, P], BF16, name=f"qtc{c}")
                    for c in range(NI)}                  # q^T per query chunk

            # ---------------- Phase 1: QKV + rotary + norms + transposes ------
            # K/V for t-tile i and Q for t-tile i-1 per iteration: Q trails
            # one tile so the PE starts on K/V as soon as the first weight
            # chunks land instead of waiting for all three weight matrices.
            # Both phases' SBUF pools coexist (no SBUF-reuse barrier at the
            # phase boundary); only the PSUM pool is swapped, so phase-2's
            # first matmuls wait just for phase-1's last PSUM readers, not
            # for every engine to drain.
            with (
                tc.tile_pool(name="wqkv", bufs=1) as wpool,
                tc.tile_pool(name="ph1", bufs=3) as ph1,
                tc.tile_pool(name="ph1t", bufs=2) as ph1t,
                tc.tile_pool(name="ph2w", bufs=1) as ph2w,
                tc.tile_pool(name="ph2", bufs=2) as ph2,
            ):
              with tc.tile_pool(name="pp1", bufs=1, space="PSUM") as pp1:
                wq_sb = wpool.tile([P, ND, GD], BF16)
                wk_sb = wpool.tile([P, ND, GD], BF16)
                wv_sb = wpool.tile([P, ND, GD], BF16)

                acts = {}

                # DMA queue split: weights on the sync queue, x tiles on the
                # ACT queue, cos/sin (+ones) on the GpSimd SWDGE -- one queue
                # can't sustain all 23MB of phase-1 input without starving
                # the x tiles behind the weight bulk.
                def load_acts(tt):
                    tsl = slice(tt * P, (tt + 1) * P)
                    if tt == 0:
                        # Separate TILES (deps are tracked per tile, not per
                        # slice): the first K matmul's LDWEIGHTS only waits
                        # for the small head tile, not the full 512KB load.
                        xh = ph1.tile([P, 2, P], BF16, tag="x0h", bufs=1,
                                      name="xtile0h")
                        # first thing on the sync queue: the first matmul's
                        # stationary data
                        nc.sync.dma_start(out=xh, in_=xt_v[:, :2, tsl])
                        xt_t = ph1.tile([P, ND - 2, P], BF16, tag="x0t",
                                        bufs=1, name="xtile0t")
                        nc.scalar.dma_start(out=xt_t, in_=xt_v[:, 2:, tsl])

                        def xf(d, xh=xh, xt_t=xt_t):
                            return xh[:, d, :] if d < 2 else xt_t[:, d - 2, :]
                    else:
                        xtile = ph1.tile([P, ND, P], BF16, tag="xtile",
                                         name=f"xtile{tt}")
                        nc.scalar.dma_start(out=xtile, in_=xt_v[:, :, tsl])

                        def xf(d, xtile=xtile):
                            return xtile[:, d, :]
                    ctile = ph1.tile([P, GD], F32, tag="ctile", name=f"ctile{tt}")
                    stile = ph1.tile([P, GD], F32, tag="stile", name=f"stile{tt}")
                    nc.gpsimd.dma_start(out=ctile,
                                        in_=cosb[tt * P:(tt + 1) * P, :])
                    nc.gpsimd.dma_start(out=stile,
                                        in_=sinb[tt * P:(tt + 1) * P, :])
                    acts[tt] = (xf, ctile, stile)

                # First K/V weight chunks land before the bulk x tile so the
                # PE's first matmul isn't gated on the largest DMA; the rest
                # is d-interleaved so each K/V contraction step can begin as
                # its chunks arrive.  Q chunks trail by design.  The ones
                # tile (first needed in phase 2) loads after the hot path.
                load_acts(0)
                nc.sync.dma_start(out=wk_sb[:, 0, :], in_=wkt_v[:, 0, :])
                nc.sync.dma_start(out=wv_sb[:, 0, :], in_=wvt_v[:, 0, :])
                for d in range(ND):
                    if d > 0:
                        # first few K/V chunks ride the near-idle gpsimd
                        # queue so the startup matmuls aren't gated on one
                        # queue draining the whole weight bulk
                        wq_ = nc.gpsimd if d < 4 else nc.sync
                        wq_.dma_start(out=wk_sb[:, d, :], in_=wkt_v[:, d, :])
                        wq_.dma_start(out=wv_sb[:, d, :], in_=wvt_v[:, d, :])
                    nc.sync.dma_start(out=wq_sb[:, d, :], in_=wqt_v[:, d, :])
                nc.gpsimd.dma_start(out=ones_t, in_=onesd[:, :])

                def qk_dve(which, src, tt):
                    """Rotary + rms stats on DVE/ACT, then DMA-transpose the
                    rotated bf16 tile into the resident q^T/k^T store."""
                    _, ctile, stile = acts[tt]
                    u = ph1t.tile([P, HG, 2, 64], F32, tag="u")
                    w = ph1t.tile([P, HG, 2, 64], F32, tag="w")
                    nc.vector.tensor_mul(u.rearrange("p h x y -> p (h x y)"), src, ctile)
                    nc.vector.tensor_mul(w.rearrange("p h x y -> p (h x y)"), src, stile)
                    rot = ph1t.tile([P, HG, P], BF16, tag=which + "rot")
                    r3 = rot.rearrange("p h (x y) -> p h x y", x=2)
                    nc.vector.tensor_add(r3[:, :, 0, :], u[:, :, 0, :], w[:, :, 1, :])
                    nc.vector.tensor_sub(r3[:, :, 1, :], u[:, :, 1, :], w[:, :, 0, :])

                    ms = ph1t.tile([P, HG], F32, tag=which + "ms")
                    sq = ph1t.tile([P, HG, P], BF16, tag="sq")
                    rflat = rot.rearrange("p h c -> p (h c)")
                    nc.vector.tensor_mul(
                        sq.rearrange("p h c -> p (h c)"), rflat, rflat)
                    nc.vector.reduce_sum(out=ms, in_=sq,
                                         axis=mybir.AxisListType.X)
                    srt = ph1t.tile([P, HG], F32, tag=which + "srt")
                    if which == "q":
                        nc.scalar.activation(out=srt, in_=ms, func=AF.Sqrt,
                                             scale=1.0 / HEAD_DIM,
                                             bias=eps_q[:, 0:1])
                        rstd = ph1t.tile([P, HG], F32, tag="qrstd")
                        nc.vector.reciprocal(out=rstd, in_=srt)
                        for h in range(HG):
                            nc.vector.tensor_scalar_mul(
                                rot[:, h, :], rot[:, h, :], rstd[:, h:h + 1])
                    else:
                        # 0.12/rms(k) folded in: 0.12/sqrt(z) = 1/sqrt(z/0.0144)
                        nc.scalar.activation(
                            out=srt, in_=ms, func=AF.Sqrt,
                            scale=1.0 / (HEAD_DIM * SCALE * SCALE),
                            bias=eps_k[:, 0:1])
                        nc.vector.reciprocal(out=rk_t[tt], in_=srt)
                    return rot

                def qk_transpose(which, rot, tt):
                    def dst(h):
                        if which == "k":
                            return kt_t[tt][:, h, :]
                        return qt_c[tt // 4][:, tt % 4, h, :]
                    for h in range(HG):
                        ptr = pp1.tile([P, P], BF16, tag="ptr", bufs=2)
                        nc.tensor.transpose(ptr, rot[:, h, :], identity)
                        nc.vector.tensor_copy(out=dst(h), in_=ptr)

                # Software pipeline: per iteration i the PE runs K/V(i) then
                # Q(i-1), then transposes k(i) (whose DVE chain overlapped
                # the Q matmuls) and q(i-2) (chain finished an iteration ago).
                qrots = {}
                for i in range(NT + 2):
                    if 2 <= i:
                        qk_transpose("q", qrots.pop(i - 2), i - 2)
                    if i < NT:
                        if i + 1 < NT:
                            load_acts(i + 1)
                        xf = acts[i][0]
                        ps_k = pp1.tile([P, GD], F32, tag="psk", bufs=2)
                        ps_v = pp1.tile([P, GD], F32, tag="psv", bufs=2)
                        for d in range(ND):
                            st, sp = d == 0, d == ND - 1
                            nc.tensor.matmul(ps_k, xf(d), wk_sb[:, d, :],
                                             start=st, stop=sp)
                            nc.tensor.matmul(ps_v, xf(d), wv_sb[:, d, :],
                                             start=st, stop=sp)
                        if has_qkv_bias:
                            kb = ph1t.tile([P, GD], F32, tag="kb")
                            nc.vector.tensor_add(kb, ps_k, bias_b[:, 1, :])
                            nc.vector.tensor_add(v_sb[:, i, :], ps_v,
                                                 bias_b[:, 2, :])
                            src_k = kb
                        else:
                            # GpSimd can't read PSUM; ACT is idle in phase 1
                            nc.scalar.activation(out=v_sb[:, i, :], in_=ps_v,
                                                 func=AF.Copy)
                            src_k = ps_k
                        krot = qk_dve("k", src_k, i)
                    if 1 <= i <= NT:
                        tt = i - 1
                        xf = acts[tt][0]
                        ps_q = pp1.tile([P, GD], F32, tag="psq", bufs=2)
                        for d in range(ND):
                            nc.tensor.matmul(ps_q, xf(d), wq_sb[:, d, :],
                                             start=d == 0, stop=d == ND - 1)
                        if has_qkv_bias:
                            qb = ph1t.tile([P, GD], F32, tag="qb")
                            nc.vector.tensor_add(qb, ps_q, bias_b[:, 0, :])
                            src_q = qb
                        else:
                            src_q = ps_q
                        qrots[tt] = qk_dve("q", src_q, tt)
                    if i < NT:
                        qk_transpose("k", krot, i)

              # -------------- Phase 2: attention + projection ---------------
              with tc.tile_pool(name="pp2", bufs=1, space="PSUM") as pp2:
                wp_sb = ph2w.tile([P, HG, DIM], BF16)

                def ysl(y_tiles, co, it):
                    t = y_tiles[co // 2]
                    if isinstance(t, dict):   # final pair: per-it-chunk tiles
                        return t[it][:, co % 2, :]
                    return t[:, co % 2, it * P:(it + 1) * P]

                def proj_half(ic, y_tiles, half, drain_act=False):
                    for it in (half, 2 + half):
                        for dc in range(4):
                            ps_o = pp2.tile([P, 512], F32, tag="po", bufs=2)
                            for co in range(HG):
                                nc.tensor.matmul(
                                    ps_o,
                                    ysl(y_tiles, co, it),
                                    wp_sb[:, co, dc * 512:(dc + 1) * 512],
                                    start=co == 0, stop=co == HG - 1)
                            o_sb = ph2.tile([P, 512], F32, tag="o", bufs=3)
                            if drain_act:
                                # final flush: DVE is busy with the last
                                # normalize; ACT is idle by now
                                nc.scalar.activation(out=o_sb, in_=ps_o,
                                                     func=AF.Copy)
                            else:
                                nc.vector.tensor_copy(out=o_sb, in_=ps_o)
                            nc.sync.dma_start(
                                out=out[(4 * ic + it) * P:(4 * ic + it + 1) * P,
                                        dc * 512:(dc + 1) * 512],
                                in_=o_sb)

                prev = None
                for ic in range(NI):
                    if ic == 0:
                        nc.sync.dma_start(out=wp_sb, in_=wpt_v)
                    qT = {h: qt_sb[:, 4 * ic:4 * ic + 4, h, :] for h in range(HG)}
                    # one y tile per head PAIR: deps are tracked per tile, so
                    # proj's LDWEIGHTS on heads 0-1 isn't gated on the last
                    # pair's normalize chain
                    y_tiles = {}
                    nj = 4 * (ic + 1)
                    # Heads in interleaved pairs with scores+exp prefetched one
                    # j-step ahead: by the time the PE reaches l/pV of step j,
                    # the exp (and diagonal-block mask) of step j has had four
                    # matmuls' worth of time to finish.  Half of the previous
                    # chunk's projection is emitted at each pair boundary so
                    # the PE chews on it exactly where it would otherwise wait
                    # for the pair's PSUM accumulators to drain.
                    for hp in range(HG // 2):
                        hs = (2 * hp, 2 * hp + 1)
                        ps_ys = {h: pp2.tile([P, 512], F32, tag="py", bufs=2,
                                             name=f"ps_y{h}") for h in hs}
                        ps_ls = {h: pp2.tile([P, 512], F32, tag="pl", bufs=2,
                                             name=f"ps_l{h}") for h in hs}

                        def sc_exp(jt):
                            pt = ph2.tile([P, 2, 512], BF16, tag="p", bufs=3)
                            for hi, h in enumerate(hs):
                                ps_s = pp2.tile([P, 512], F32, tag="ps", bufs=2)
                                nc.tensor.matmul(
                                    ps_s, kt_sb[:, jt, h, :],
                                    qT[h], start=True, stop=True)
                                nc.scalar.activation(
                                    out=pt[:, hi, :], in_=ps_s, func=AF.Exp,
                                    scale=rk_sb[:, jt, h:h + 1])
                                if jt >= 4 * ic:
                                    nc.gpsimd.affine_select(
                                        out=pt[:, hi, :], in_=pt[:, hi, :],
                                        pattern=[[1, 512]], channel_multiplier=-1,
                                        base=-P * (jt - 4 * ic),
                                        compare_op=mybir.AluOpType.is_ge, fill=0.0)
                            return pt

                        pts = sc_exp(0)
                        for jt in range(nj):
                            nxt = None
                            if jt + 1 < nj:
                                nxt = sc_exp(jt + 1)
                            st, sp = jt == 0, jt == nj - 1
                            for hi, h in enumerate(hs):
                                nc.tensor.matmul(ps_ls[h], ones_t,
                                                 pts[:, hi, :], start=st, stop=sp)
                                nc.tensor.matmul(
                                    ps_ys[h], v_sb[:, jt, h * P:(h + 1) * P],
                                    pts[:, hi, :], start=st, stop=sp)
                            pts = nxt
                            # Previous chunk's projection lands after this
                            # pair's fourth j-step: the PE runs it while this
                            # pair streams, and the previous pair's
                            # recip+normalize chain (DVE, ~4us) has three full
                            # j-steps of slack before proj's LDWEIGHTS needs
                            # the chunk's last y_sb slice.
                            if jt == 3 and prev is not None:
                                proj_half(prev[0], prev[1], hp)
                        # Fast ACT copies release all four PSUM banks within
                        # ~1us of the pair's last matmul; the slow reciprocal
                        # and the normalize run SBUF-side on DVE with a full
                        # pair of slack before y_sb is consumed by proj.
                        # Quick bf16 drains free all four PSUM banks fast:
                        # y on DVE (idle at pair end), l on ACT; the slow
                        # reciprocal + normalize run SBUF-side afterwards,
                        # off every matmul's critical path (per-pair y tiles
                        # + proj placement give them ~2 j-steps of slack).
                        linv = ph2.tile([P, 2, 512], BF16, tag="linv")
                        last = ic == NI - 1 and hp == HG // 2 - 1
                        if last:
                            # Nothing follows: normalize straight from PSUM,
                            # in four per-it-chunk tiles emitted in the proj
                            # flush's consumption order (it = 0,2,1,3), so
                            # the flush starts after the first 128-col chunk
                            # instead of after the whole recip chain.
                            yd = {}
                            with nc.allow_low_precision(reason="bf16 y"):
                                for c in (0, 2, 1, 3):
                                    yt = ph2.tile([P, 2, P], BF16, tag="yf",
                                                  bufs=4, name=f"yf{c}")
                                    yd[c] = yt
                                    sl = slice(c * P, (c + 1) * P)
                                    for hi, h in enumerate(hs):
                                        nc.vector.reciprocal(
                                            out=linv[:, hi, sl],
                                            in_=ps_ls[h][:, sl])
                                        nc.vector.tensor_mul(
                                            yt[:, hi, :], ps_ys[h][:, sl],
                                            linv[:, hi, sl])
                            y_tiles[hp] = yd
                        else:
                            y_raw = ph2.tile([P, 2, 512], BF16, tag="y_raw")
                            l_sb = ph2.tile([P, 2, 512], BF16, tag="l_sb")
                            for hi, h in enumerate(hs):
                                nc.vector.tensor_copy(out=y_raw[:, hi, :],
                                                      in_=ps_ys[h])
                                nc.scalar.activation(out=l_sb[:, hi, :],
                                                     in_=ps_ls[h],
                                                     func=AF.Copy)
                            # fine-grained: four per-it-chunk y tiles in the
                            # proj flush's consumption order, so proj's
                            # per-tile waits cover one 128-col recip, not
                            # the whole chain
                            yd = {}
                            with nc.allow_low_precision(
                                    reason="1/l and y at bf16; y is consumed "
                                           "as bf16 anyway"):
                                for c in (0, 2, 1, 3):
                                    yt = ph2.tile([P, 2, P], BF16,
                                                  tag=f"y{hp}c", bufs=8,
                                                  name=f"y{ic}_{hp}_{c}")
                                    yd[c] = yt
                                    sl = slice(c * P, (c + 1) * P)
                                    for hi, h in enumerate(hs):
                                        nc.vector.reciprocal(
                                            out=linv[:, hi, sl],
                                            in_=l_sb[:, hi, sl])
                                        nc.vector.tensor_mul(
                                            yt[:, hi, :],
                                            y_raw[:, hi, sl],
                                            linv[:, hi, sl])
                            y_tiles[hp] = yd
                    prev = (ic, y_tiles)
                proj_half(prev[0], prev[1], 0, drain_act=True)
                proj_half(prev[0], prev[1], 1, drain_act=True)
    _split_excess_waits(nc)
    return nc


_NC_CACHE = {}
_RUN_KWARGS = {}      # test harness hook: e.g. {"trace": True}
_LAST_RESULT = None   # BassKernelResults of the most recent run


def _rotary_tables():
    freq = (1.0 / 1024.0) ** np.linspace(0.0, 1.0, HEAD_DIM // 4, dtype=np.float32)
    freq = np.concatenate([freq, np.zeros(HEAD_DIM // 4, np.float32)])
    theta = np.arange(T, dtype=np.float32)[:, None] * freq[None, :]     # [T, 64]
    cos = np.cos(theta).astype(np.float32)
    sin = np.sin(theta).astype(np.float32)
    cosb = np.tile(np.concatenate([cos, cos], axis=1), (1, HG))          # [T, 512]
    sinb = np.tile(np.concatenate([sin, sin], axis=1), (1, HG))
    return np.ascontiguousarray(cosb), np.ascontiguousarray(sinb)


def kernel(x, Wq, bq, Wk, bk, Wv, bv, Wp, bp):
    import ml_dtypes
    BF = ml_dtypes.bfloat16

    x = np.asarray(x, np.float32)
    Wq, Wk, Wv, Wp = (np.asarray(a, np.float32) for a in (Wq, Wk, Wv, Wp))
    bq, bk, bv, bp = (np.asarray(a, np.float32) for a in (bq, bk, bv, bp))

    has_bias = bool(np.any(bq) or np.any(bk) or np.any(bv))
    if has_bias not in _NC_CACHE:
        _NC_CACHE[has_bias] = _build_nc(has_bias)
    nc = _NC_CACHE[has_bias]

    cosb, sinb = _rotary_tables()
    in_maps = []
    for c in range(NCORES):
        b, g = divmod(c, NCORES // B)
        sl = slice(g * GD, (g + 1) * GD)
        m = {
            "xt": np.ascontiguousarray(x[b].T.astype(BF)),
            "wqt": np.ascontiguousarray(Wq[sl, :].T.astype(BF)),
            "wkt": np.ascontiguousarray(Wk[sl, :].T.astype(BF)),
            "wvt": np.ascontiguousarray(Wv[sl, :].T.astype(BF)),
            "wpt": np.ascontiguousarray(Wp[:, sl].T.astype(BF)),
            "cosb": cosb,
            "sinb": sinb,
            "onesd": np.ones((P, P), BF),
        }
        if has_bias:
            m["bq"] = np.ascontiguousarray(bq[sl])
            m["bk"] = np.ascontiguousarray(bk[sl])
            m["bv"] = np.ascontiguousarray(bv[sl])
        in_maps.append(m)

    res = run_bass_kernel_spmd(nc, in_maps, list(range(NCORES)), **_RUN_KWARGS)
    global _LAST_RESULT
    _LAST_RESULT = res
    out = np.zeros((B, T, DIM), np.float32)
    for c in range(NCORES):
        out[c // (NCORES // B)] += res.results[c]["out"]
    out += bp[None, None, :]
    return out
